# revision 1
# baseline (speedup 1.0000x reference)
"""Trainium2 Bass kernel for nn_CausalSelfAttention_37417755083187.

Full-input contract: kernel(**inputs) takes the unsharded inputs and returns
the full [B, T, C] output.  Internally shards over 8 NeuronCores as
(2 batches) x (4 head-groups of 4 heads); each core computes a partial
projection output (row-split Wproj) and the host sums the 4 partials/batch.
"""

import sys

sys.path.insert(0, "/opt/trn_rl_repo")

from contextlib import ExitStack

import numpy as np

import concourse.bass as bass
import concourse.mybir as mybir
import concourse.tile as tile
from concourse import bacc
from concourse.alu_op_type import AluOpType as alu

F32 = mybir.dt.float32
BF16 = mybir.dt.bfloat16
AF = mybir.ActivationFunctionType

# Problem constants (hardcoded per harness contract)
B, T, C = 2, 2048, 2048
NH = 16
HD = 128
D2 = HD // 2  # 64
GATE = 32
EPS = 1e-6
N_CORES = 8
N_GROUPS = 4          # head-groups (tensor parallel)
NHC = NH // N_GROUPS  # heads per core = 4


def build_nc(T_=T, C_=C, NHC_=NHC, num_devices=N_CORES):
    """Build the Bass program for one core (SPMD: all cores run this)."""
    NQ = NHC_ * HD          # per-core qkv width
    TT = T_ // 128          # token tiles
    CT = C_ // 128          # channel tiles
    NCH = T_ // 512         # 512-row chunks (x transpose / t-tile grouping)
    TQC = T_ // 512         # tq chunks in attention
    VW = 130                # per-head v width: 128 v + 1 ones + 1 pad

    nc = bacc.Bacc(
        "TRN2",
        target_bir_lowering=False,
        debug=False,
        enable_asserts=False,
        num_devices=num_devices,
    )

    x_d = nc.dram_tensor("x_s", [T_, C_], F32, kind="ExternalInput").ap()
    ve_d = nc.dram_tensor("ve_s", [T_, NQ], F32, kind="ExternalInput").ap()
    cos_d = nc.dram_tensor("cos_s", [T_, D2], F32, kind="ExternalInput").ap()
    sin_d = nc.dram_tensor("sin_s", [T_, D2], F32, kind="ExternalInput").ap()
    wq_d = nc.dram_tensor("wq_s", [C_, NQ], F32, kind="ExternalInput").ap()
    wk_d = nc.dram_tensor("wk_s", [C_, NQ], F32, kind="ExternalInput").ap()
    wv_d = nc.dram_tensor("wv_s", [C_, NQ], F32, kind="ExternalInput").ap()
    wg_d = nc.dram_tensor("wg_s", [GATE, NHC_], F32, kind="ExternalInput").ap()
    wp_d = nc.dram_tensor("wp_s", [NQ, C_], F32, kind="ExternalInput").ap()
    out_d = nc.dram_tensor("out_s", [T_, C_], F32, kind="ExternalOutput").ap()

    with ExitStack() as ctx:
        tc = ctx.enter_context(tile.TileContext(nc))
        pp = ctx.enter_context(tc.tile_pool(name="persist", bufs=1))
        dpool = ctx.enter_context(tc.tile_pool(name="dram", bufs=1, space="DRAM"))

        qT = pp.tile([128, NHC_, T_], BF16, name="qT")   # [d, h, t]
        kT = pp.tile([128, NHC_, T_], BF16, name="kT")
        vext = pp.tile([128, TT, NHC_ * VW], BF16, name="vext")  # [t, tile, (h: 128v+1+pad)]
        g_all = pp.tile([128, TT, NHC_], F32, name="g_all")
        cos_bf = pp.tile([128, TT, D2], BF16, name="cos_bf")
        sin_bf = pp.tile([128, TT, D2], BF16, name="sin_bf")
        wgate_b = pp.tile([GATE, NHC_], BF16, name="wgate_b")

        xbf = dpool.tile([T_, C_], BF16, name="xbf")

        vext_v = vext.rearrange("p t (h c) -> p t h c", c=VW)

        # ---------------- Phase A: QKV / gate / RoPE / RMS ----------------
        with tc.tile_pool(name="pA", bufs=2) as pa, \
             tc.tile_pool(name="wqkv", bufs=1) as wpool, \
             tc.tile_pool(name="psA", bufs=2, space="PSUM") as psA:

            # cos/sin load + bf16 convert
            cosf = pa.tile([128, TT, D2], F32, tag="cs", bufs=2)
            nc.sync.dma_start(cosf, cos_d.rearrange("(a p) d -> p a d", p=128))
            nc.vector.tensor_copy(cos_bf, cosf)
            sinf = pa.tile([128, TT, D2], F32, tag="cs", bufs=2)
            nc.sync.dma_start(sinf, sin_d.rearrange("(a p) d -> p a d", p=128))
            nc.vector.tensor_copy(sin_bf, sinf)

            # wgate: fold the tanh(z/2) half into the weight
            wgf = pa.tile([GATE, NHC_], F32, tag="wgf", bufs=1)
            nc.sync.dma_start(wgf, wg_d)
            nc.scalar.mul(wgate_b, wgf, 0.5)

            # init vext (zeros + ones columns for the fused denominator)
            nc.vector.memset(vext, 0.0)
            nc.vector.memset(vext_v[:, :, :, 128:129], 1.0)

            # W_q/k/v: load fp32 [128, CT, NQ] in quarters, convert to bf16
            wqkv_b = []
            for wd, nm in ((wq_d, "wq_b"), (wk_d, "wk_b"), (wv_d, "wv_b")):
                wb = wpool.tile([128, CT, NQ], BF16, name=nm)
                wr = wd.rearrange("(a p) n -> p a n", p=128)
                nq = max(1, CT // 4)
                for qtr in range(CT // nq):
                    wf = pa.tile([128, nq, NQ], F32, tag="wstage", bufs=2)
                    nc.sync.dma_start(wf, wr[:, qtr * nq:(qtr + 1) * nq, :])
                    nc.scalar.copy(wb[:, qtr * nq:(qtr + 1) * nq, :], wf)
                wqkv_b.append(wb)
            wq_b, wk_b, wv_b = wqkv_b

            # stage x -> bf16 in DRAM (for the xbar transpose)
            for s in range(TT):
                xs = pa.tile([128, C_], F32, tag="xs")
                nc.sync.dma_start(xs, x_d[bass.ts(s, 128), :])
                xb = pa.tile([128, C_], BF16, tag="xb")
                nc.vector.tensor_copy(xb, xs)
                nc.sync.dma_start(xbf[bass.ts(s, 128), :], xb)

            for ch in range(NCH):
                # xT chunk: [c_part, c_tile, t(512)] via DMA xbar transpose
                xTc = pa.tile([128, CT, 512], BF16, tag="xT")
                nc.sync.dma_start_transpose(xTc, xbf[ch * 512:(ch + 1) * 512, :])

                # gate: u = (x[:, :32] @ Wg)/2 ; gate = 1 + tanh(u) via series
                for t4 in range(4):
                    t = ch * 4 + t4
                    gps = psA.tile([128, NHC_], F32, tag="g")
                    nc.tensor.matmul(gps, xTc[0:GATE, 0, bass.ts(t4, 128)],
                                     wgate_b, start=True, stop=True)
                    gu = pa.tile([128, NHC_], F32, tag="gu")
                    nc.vector.tensor_copy(gu, gps)
                    ga = pa.tile([128, NHC_], F32, tag="ga")
                    nc.vector.tensor_mul(ga, gu, gu)          # u^2
                    gb = pa.tile([128, NHC_], F32, tag="gb")
                    nc.vector.tensor_mul(gb, ga, gu)          # u^3
                    gc = pa.tile([128, NHC_], F32, tag="gc")
                    # u - u^3/3
                    nc.vector.scalar_tensor_tensor(out=gc, in0=gb, scalar=-1.0 / 3.0,
                                                   in1=gu, op0=alu.mult, op1=alu.add)
                    ge = pa.tile([128, NHC_], F32, tag="ge")
                    nc.vector.tensor_mul(ge, ga, gb)          # u^5
                    gf = pa.tile([128, NHC_], F32, tag="gf")
                    nc.vector.scalar_tensor_tensor(out=gf, in0=ge, scalar=2.0 / 15.0,
                                                   in1=gc, op0=alu.mult, op1=alu.add)
                    nc.vector.tensor_scalar_add(g_all[:, t, :], gf, 1.0)

                for t4 in range(4):
                    t = ch * 4 + t4
                    qps = psA.tile([128, NQ], F32, tag="q")
                    kps = psA.tile([128, NQ], F32, tag="k")
                    vps = psA.tile([128, NQ], F32, tag="v")
                    for c in range(CT):
                        lhs = xTc[:, c, bass.ts(t4, 128)]
                        st, sp = (c == 0), (c == CT - 1)
                        nc.tensor.matmul(qps, lhs, wq_b[:, c, :], start=st, stop=sp)
                        nc.tensor.matmul(kps, lhs, wk_b[:, c, :], start=st, stop=sp)
                        nc.tensor.matmul(vps, lhs, wv_b[:, c, :], start=st, stop=sp)

                    # ---- v epilogue: v + gate*ve, write bf16 into vext ----
                    vet = pa.tile([128, NQ], F32, tag="ve")
                    nc.sync.dma_start(vet, ve_d[bass.ts(t, 128), :])
                    for h in range(NHC_):
                        nc.vector.scalar_tensor_tensor(
                            out=vext_v[:, t, h, 0:128],
                            in0=vet[:, bass.ts(h, 128)],
                            scalar=g_all[:, t, h:h + 1],
                            in1=vps[:, bass.ts(h, 128)],
                            op0=alu.mult, op1=alu.add)

                    # ---- q/k epilogue: RoPE + RMS-norm + transpose ----
                    qkb = pa.tile([128, 2, NQ], BF16, tag="qkb")
                    nc.scalar.copy(qkb[:, 0, :], qps)
                    nc.scalar.copy(qkb[:, 1, :], kps)
                    qk4 = qkb.rearrange("p a (h x d) -> p a h x d", h=NHC_, x=2)
                    z1 = qk4[:, :, :, 0, :]
                    z2 = qk4[:, :, :, 1, :]
                    cb = cos_bf[:, t, :].unsqueeze(1).unsqueeze(1) \
                        .broadcast_to([128, 2, NHC_, D2])
                    sb = sin_bf[:, t, :].unsqueeze(1).unsqueeze(1) \
                        .broadcast_to([128, 2, NHC_, D2])
                    rot = pa.tile([128, 2, NQ], BF16, tag="rot")
                    rot4 = rot.rearrange("p a (h x d) -> p a h x d", h=NHC_, x=2)
                    t1 = pa.tile([128, 2, NHC_, D2], BF16, tag="t1")
                    t2 = pa.tile([128, 2, NHC_, D2], BF16, tag="t2")
                    nc.vector.tensor_mul(t1, z1, cb)
                    nc.vector.tensor_mul(t2, z2, sb)
                    nc.vector.tensor_add(rot4[:, :, :, 0, :], t1, t2)
                    nc.vector.tensor_mul(t1, z2, cb)
                    nc.vector.tensor_mul(t2, z1, sb)
                    nc.vector.tensor_sub(rot4[:, :, :, 1, :], t1, t2)

                    # RMS stats: per-(qk, head) sum of squares
                    sq = pa.tile([128, 2, NHC_, HD], F32, tag="sq")
                    rot_h = rot.rearrange("p a (h d) -> p a h d", h=NHC_)
                    nc.vector.tensor_mul(sq, rot_h, rot_h)
                    sums = pa.tile([128, 2, NHC_], F32, tag="sums")
                    nc.vector.reduce_sum(sums, sq, axis=mybir.AxisListType.X)
                    # q: scale = rsqrt(sum + 128*eps)  (folds mean+1/sqrt(HD))
                    nc.vector.tensor_scalar_add(sums[:, 0, :], sums[:, 0, :],
                                                float(HD) * EPS)
                    # k: scale = rsqrt(sum/128 + eps)
                    nc.vector.tensor_scalar(out=sums[:, 1, :], in0=sums[:, 1, :],
                                            scalar1=1.0 / HD, scalar2=EPS,
                                            op0=alu.mult, op1=alu.add)
                    s0 = pa.tile([128, 2, NHC_], F32, tag="s0")
                    nc.scalar.sqrt(s0, sums)
                    r0 = pa.tile([128, 2, NHC_], F32, tag="r0")
                    nc.vector.reciprocal(r0, s0)
                    # one Newton step: r = r0*(1.5 - 0.5*m*r0^2)
                    n1 = pa.tile([128, 2, NHC_], F32, tag="n1")
                    nc.vector.tensor_mul(n1, r0, r0)
                    nc.vector.tensor_mul(n1, n1, sums)
                    nc.vector.tensor_scalar(out=n1, in0=n1, scalar1=-0.5,
                                            scalar2=1.5, op0=alu.mult, op1=alu.add)
                    nc.vector.tensor_mul(r0, r0, n1)
                    for a in range(2):
                        for h in range(NHC_):
                            sl = rot[:, a, bass.ts(h, HD)]
                            nc.vector.tensor_scalar_mul(sl, sl, r0[:, a, h:h + 1])
                    nc.sync.dma_start_transpose(qT[:, :, bass.ts(t, 128)], rot[:, 0, :])
                    nc.sync.dma_start_transpose(kT[:, :, bass.ts(t, 128)], rot[:, 1, :])

        # ---------------- Phase B: attention ----------------
        with tc.tile_pool(name="pB2", bufs=1) as pb2:
            yT = pb2.tile([128, NHC_, T_], BF16, name="yT")
            yn = pb2.tile([128, TT, NQ], BF16, name="yn")
            wp_b = pb2.tile([128, NHC_, C_], BF16, name="wp_b")
            with tc.tile_pool(name="wps", bufs=1) as wps:
                wpf = wps.tile([128, NHC_, C_], F32, name="wpf")
                nc.sync.dma_start(wpf, wp_d.rearrange("(h p) c -> p h c", p=128))
                nc.scalar.copy(wp_b, wpf)

            with tc.tile_pool(name="pBw", bufs=2) as pbw, \
                 tc.tile_pool(name="psB", bufs=2, space="PSUM") as psB:
                for h in range(NHC_):
                    for jq in range(TQC):
                        n_tk = 4 * (jq + 1)
                        P_all = pbw.tile([128, TT, 512], BF16, tag="P")
                        for p in range(n_tk // 2):
                            s_ps = psB.tile([128, 2, 512], F32, tag="s")
                            for s2 in (0, 1):
                                i = 2 * p + s2
                                nc.tensor.matmul(
                                    s_ps[:, s2, :],
                                    kT[:, h, bass.ts(i, 128)],
                                    qT[:, h, bass.ts(jq, 512)],
                                    start=True, stop=True)
                            nc.scalar.activation(P_all[:, 2 * p:2 * p + 2, :],
                                                 s_ps, AF.Exp)
                            if p >= n_tk // 2 - 2:
                                # causal: keep tq - tk >= 0
                                nc.gpsimd.affine_select(
                                    out=P_all[:, 2 * p:2 * p + 2, :],
                                    in_=P_all[:, 2 * p:2 * p + 2, :],
                                    pattern=[[-128, 2], [1, 512]],
                                    compare_op=alu.is_ge,
                                    fill=0.0,
                                    base=512 * jq - 128 * 2 * p,
                                    channel_multiplier=-1)
                        for q4 in range(4):
                            tqt = 4 * jq + q4
                            y_ps = psB.tile([128, HD + 1], F32, tag="y")
                            for i in range(tqt + 1):
                                nc.tensor.matmul(
                                    y_ps,
                                    P_all[:, i, bass.ts(q4, 128)],
                                    vext_v[:, i, h, 0:HD + 1],
                                    start=(i == 0), stop=(i == tqt))
                            ycp = pbw.tile([128, HD + 1], F32, tag="ycp")
                            nc.vector.tensor_copy(ycp, y_ps)
                            nc.gpsimd.normalize_recip(
                                out_ap=yn[:, tqt, bass.ts(h, HD)],
                                in_ap=ycp[:, 0:HD],
                                denom_ap=ycp[:, HD:HD + 1])
                for t in range(TT):
                    nc.sync.dma_start_transpose(yT[:, :, bass.ts(t, 128)],
                                                yn[:, t, :])

            # ---------------- Phase C: projection ----------------
            with tc.tile_pool(name="pC", bufs=2) as pc, \
                 tc.tile_pool(name="psC", bufs=2, space="PSUM") as psC:
                for t in range(TT):
                    ob = pc.tile([128, C_], F32, tag="ob")
                    for c4 in range(C_ // 512):
                        o_ps = psC.tile([128, 512], F32, tag="o")
                        for h in range(NHC_):
                            nc.tensor.matmul(o_ps, yT[:, h, bass.ts(t, 128)],
                                             wp_b[:, h, bass.ts(c4, 512)],
                                             start=(h == 0), stop=(h == NHC_ - 1))
                        dst = ob[:, bass.ts(c4, 512)]
                        if c4 % 2 == 0:
                            nc.scalar.copy(dst, o_ps)
                        else:
                            nc.vector.tensor_copy(dst, o_ps)
                    nc.sync.dma_start(out_d[bass.ts(t, 128), :], ob)

    nc.compile()
    return nc


def shard_inputs(inputs):
    """Full inputs -> list of 8 per-core input maps."""
    x = np.asarray(inputs["x"], np.float32)
    ve = np.asarray(inputs["ve"], np.float32)
    cos = np.asarray(inputs["cos"], np.float32).reshape(T, D2)
    sin = np.asarray(inputs["sin"], np.float32).reshape(T, D2)
    wq = np.asarray(inputs["Wq"], np.float32)
    wk = np.asarray(inputs["Wk"], np.float32)
    wv = np.asarray(inputs["Wv"], np.float32)
    wg = np.asarray(inputs["Wgate"], np.float32)
    wp = np.asarray(inputs["Wproj"], np.float32)

    NQ = NHC * HD
    maps = []
    for core in range(N_CORES):
        b, g = divmod(core, N_GROUPS)
        sl = slice(g * NQ, (g + 1) * NQ)
        maps.append({
            "x_s": np.ascontiguousarray(x[b]),
            "ve_s": np.ascontiguousarray(ve[b][:, sl]),
            "cos_s": np.ascontiguousarray(cos),
            "sin_s": np.ascontiguousarray(sin),
            "wq_s": np.ascontiguousarray(wq[:, sl]),
            "wk_s": np.ascontiguousarray(wk[:, sl]),
            "wv_s": np.ascontiguousarray(wv[:, sl]),
            "wg_s": np.ascontiguousarray(wg[:, g * NHC:(g + 1) * NHC]),
            "wp_s": np.ascontiguousarray(wp[sl, :]),
        })
    return maps


_NC_CACHE = {}


def _get_nc():
    if "nc" not in _NC_CACHE:
        _NC_CACHE["nc"] = build_nc()
    return _NC_CACHE["nc"]


def kernel(**inputs) -> np.ndarray:
    from concourse.bass_utils import run_bass_kernel_spmd

    nc = _get_nc()
    in_maps = shard_inputs(inputs)
    res = run_bass_kernel_spmd(nc, in_maps, list(range(N_CORES)))
    out = np.zeros((B, T, C), np.float32)
    for core in range(N_CORES):
        b = core // N_GROUPS
        out[b] += res.results[core]["out_s"]
    return out
